# revision 20
# baseline (speedup 1.0000x reference)
"""Trainium2 Bass kernel: 2048-point Hadamard transform.

out = (value @ H2048) * 2^-5.5,  value: (32768, 2048) f32, H2048 = Sylvester
Hadamard (symmetric, +-1) == the `weight` input.

Factorization: H2048 = H128 (x) H16 over n = 16*i + k, m = 16*j + l:
    out[t, 16j+l] = sum_k H16[k,l] * (sum_i V[t,16i+k] * H128[i,j])

Zero on-device transposes:
  * Host pre-shuffles input to va[i, (t,k)] (i on partitions), bf16.
  * Stage A is DATA-stationary: lhsT = va chunk [i, (t8,k)], rhs = H128s
    -> psA[(t8,k), j]: the layout change falls out of lhsT.T @ rhs for free.
  * Stage B is H-stationary with block-diagonal R1 = I8 (x) H16 acting on
    partitions (t8,k) -> psB[(t8,l), (c,j)].
  * Host un-shuffles the [(t8,l), (it,c,j)] output back to [t, m].

Scales: H128s = H128 * 2^-3 (exact bf16), R1 = (I8 (x) H16) * 2^-2.5;
product = 2^-5.5 = 1/sqrt(2048).

walrus in this container rejects any instruction carrying more than ONE
sync wait (one NEURON_ISA_TPB_EVENTS slot per instruction), while Tile
freely emits several (slot-reuse WAR/WAW + cross-engine RAW on one inst,
and the kernel-tail drain waits on every live semaphore). _legalize_waits
post-processes the BIR JSON: every excess wait moves onto an injected
same-engine EventSemaphore placed immediately before the instruction --
the engine stalls for the same conditions in sequence, so the schedule
semantics are unchanged.

Sharding: data-parallel on tokens across 8 cores (4096 tokens each);
bf16 in / bf16 out DMA (32 MB per core), host casts f32<->bf16.
PSUM evacuation is split DVE (stage A) / ACT (stage B).
"""

import json
import sys

import numpy as np
import ml_dtypes

import concourse.bass as bass
import concourse.mybir as mybir
import concourse.tile as tile
from concourse.bass import ts, _add_dep_helper as add_dep
from concourse.bass_utils import run_bass_kernel_spmd

N_CORES = 8
T_FULL = 32768
N = 2048
T_CORE = T_FULL // N_CORES  # 4096
P = 128
N_TILES = T_CORE // P  # 32
# Variable DMA granularity (in 512KB token tiles): small chunks at the
# edges for fast ramp-in/ramp-out, 4MB chunks mid-kernel for efficiency.
LOAD_GRPS = [1, 1, 2, 4, 8, 8, 8]
STORE_GRPS = [4, 4, 4, 4, 4, 4, 4, 2, 1, 1]
CW = 2 * P  # consts width appended to the first load

F32 = mybir.dt.float32
BF16 = mybir.dt.bfloat16
NP_BF16 = ml_dtypes.bfloat16

BASS_RAN = {"ok": False}  # introspection for test.py


def _sylvester(n: int) -> np.ndarray:
    H = np.array([[1.0]], dtype=np.float64)
    while H.shape[0] < n:
        H = np.block([[H, H], [H, -H]])
    return H


def _host_consts() -> np.ndarray:
    H128s = _sylvester(128) * 2.0**-3
    R1 = np.kron(np.eye(8), _sylvester(16)) * 2.0**-2.5
    return np.concatenate([H128s, R1], axis=1).astype(NP_BF16)  # [128, 256]


def _legalize_waits(bir: bytes) -> bytes:
    """Split multi-wait instructions for walrus's 1-wait-per-inst limit.

    Each excess wait moves to an injected same-engine EventSemaphore right
    before the instruction; the engine blocks on the same conditions in
    the same order, so this is semantics-preserving.
    """
    j = json.loads(bir)
    n = 0
    for f in j["functions"]:
        for b in f["blocks"]:
            out = []
            for i in b["instructions"]:
                si = i.get("sync_info")
                if si and si.get("on_wait") and len(si["on_wait"]) > 1:
                    waits = si["on_wait"]
                    for w in waits[:-1]:
                        n += 1
                        out.append(
                            {
                                "debug": i.get("debug", 0),
                                "engine": i["engine"],
                                "ins": [],
                                "outs": [],
                                "name": f"lgw-{n}-{i['name']}",
                                "opcode": "EventSemaphore",
                                "sync_info": {"on_update": [], "on_wait": [w]},
                            }
                        )
                    si["on_wait"] = [waits[-1]]
                out.append(i)
            b["instructions"] = out
    return json.dumps(j).encode()


def build_bass(t_core: int = T_CORE) -> bass.Bass:
    n_tiles = t_core // P
    assert sum(LOAD_GRPS) == n_tiles and sum(STORE_GRPS) == n_tiles
    load_start = {}
    s = 0
    for k, w in enumerate(LOAD_GRPS):
        load_start[s] = (k, w)
        s += w
    store_start = {}
    s = 0
    for k, w in enumerate(STORE_GRPS):
        store_start[s] = (k, w)
        s += w
    max_lw = max(LOAD_GRPS[1:])
    max_sw = max(STORE_GRPS)

    nc = bass.Bass()
    # the first load carries the consts in its last CW columns
    va_p = nc.declare_dram_parameter(
        "va", [P, t_core * 16 + CW], BF16, isOutput=False
    )
    out_p = nc.declare_dram_parameter("out", [P, t_core * 16], BF16, isOutput=True)

    with tile.TileContext(nc) as tc:
        with (
            tc.tile_pool(name="va0", bufs=1) as va0pool,
            tc.tile_pool(name="va", bufs=2) as vapool,
            tc.tile_pool(name="vt2", bufs=4) as vtpool,
            tc.tile_pool(name="og", bufs=3) as ogpool,
            tc.tile_pool(name="pa", bufs=2, space="PSUM") as papool,
            tc.tile_pool(name="pb", bufs=2, space="PSUM") as pbpool,
        ):
            H128s = R1 = None
            VA = OG = None
            va_base = og_base = 0
            vt2_of = {}
            pe_prev = [None]

            def pe_chain(ins):
                # pin PE issue order (software pipeline: stage A of iter i
                # before stage B of iter i-1) without adding semaphores
                if pe_prev[0] is not None:
                    add_dep(ins.ins, pe_prev[0].ins, sync=False)
                pe_prev[0] = ins

            def emit_stage_b(j):
                # stage B for tile j: psB[(t8,l), (c,j)] = R1.T @ vt2
                _, sw_ = store_start[og_base_of[j]]
                ob = (j - og_base_of[j]) * N
                OGj = og_of[j]
                vt2j = vt2_of.pop(j)
                for half in range(2):
                    pb = pbpool.tile([P, 2, 512], F32, tag="pb")  # 2 banks
                    for e in range(2):
                        gq = half * 2 + e
                        mm = nc.tensor.matmul(
                            pb[:, e],
                            R1,
                            vt2j[:, ts(gq, 4), :],
                            start=True,
                            stop=True,
                        )
                        pe_chain(mm)
                    # evacuate + cast (ACT)
                    nc.scalar.copy(
                        out=OGj[:, ob + half * 1024 : ob + (half + 1) * 1024],
                        in_=pb,
                    )
                if j == og_base_of[j] + sw_ - 1:
                    # store on the ACT HWDGE ring; loads use the SP ring
                    nc.scalar.dma_start(
                        out=out_p[:, og_base_of[j] * N : (og_base_of[j] + sw_) * N],
                        in_=OGj[:, 0 : sw_ * N],
                    )

            og_of = {}
            og_base_of = {}
            for it in range(n_tiles):
                if it in load_start:
                    k, w = load_start[it]
                    va_base = it
                    if k == 0:
                        fw0 = w * N
                        VA = va0pool.tile([P, fw0 + CW], BF16, tag="va0")
                        nc.sync.dma_start(out=VA, in_=va_p[:, 0 : fw0 + CW])
                        H128s = VA[:, fw0 : fw0 + P]
                        R1 = VA[:, fw0 + P : fw0 + 2 * P]
                    else:
                        VA = vapool.tile([P, max_lw * N], BF16, tag="va")
                        nc.sync.dma_start(
                            out=VA[:, 0 : w * N],
                            in_=va_p[:, CW + it * N : CW + (it + w) * N],
                        )
                if it in store_start:
                    _, sw = store_start[it]
                    og_base = it
                    OG = ogpool.tile([P, max_sw * N], BF16, tag="og")
                og_of[it] = OG
                og_base_of[it] = og_base
                base = (it - va_base) * N
                vt2 = vtpool.tile([P, 16, P], BF16, tag="vt2")
                vt2_of[it] = vt2
                # stage A: psA[(t8,k), j] = sum_i va[i,(t8,k)] H128s[i,j]
                for h in range(2):
                    pa = papool.tile([P, 8, P], F32, tag="pa")  # 2 banks
                    for cc in range(8):
                        c = h * 8 + cc
                        mm = nc.tensor.matmul(
                            pa[:, cc],
                            VA[:, base + c * P : base + (c + 1) * P],
                            H128s,
                            start=True,
                            stop=True,
                        )
                        pe_chain(mm)
                    # evacuate + cast f32->bf16 (DVE)
                    nc.vector.tensor_copy(
                        out=vt2[:, h * 8 : (h + 1) * 8, :], in_=pa
                    )
                if it > 0:
                    emit_stage_b(it - 1)
            emit_stage_b(n_tiles - 1)

    # walrus 1-wait workaround: serve a legalized BIR to every consumer
    legal = _legalize_waits(mybir.module_to_json_bytes(nc.m))
    nc.to_json_bytes = lambda: legal
    return nc


def scan_multiwait(nc) -> list:
    bad = []
    for b in nc.m.functions[0].blocks:
        for i in b.instructions:
            si = i.sync_info
            if si and si.on_wait and len(si.on_wait) > 1:
                bad.append(
                    (
                        i.name,
                        type(i).__name__,
                        str(i.engine),
                        [(w.ant_name, w.wait_value) for w in si.on_wait],
                    )
                )
    return bad


def _host_shuffle_in(v: np.ndarray, consts: np.ndarray) -> np.ndarray:
    # [t, 16i+k] f32 -> [i, t*16+k] bf16 (i on partitions); consts spliced
    # in after the first load group's columns
    t = v.shape[0]
    fw0 = LOAD_GRPS[0] * N
    vb = v.astype(NP_BF16)
    va = vb.reshape(t, 128, 16).transpose(1, 0, 2).reshape(128, t * 16)
    return np.ascontiguousarray(
        np.concatenate([va[:, :fw0], consts, va[:, fw0:]], axis=1)
    )


def _host_unshuffle_out(o: np.ndarray, t_core: int) -> np.ndarray:
    # [t8*16+l, it*2048 + c*128 + j] -> [it*128 + c*8 + t8, 16j+l] f32
    n_tiles = t_core // 128
    arr = np.asarray(o).astype(np.float32).reshape(8, 16, n_tiles, 16, 128)
    return arr.transpose(2, 3, 0, 4, 1).reshape(t_core, 2048)


_CACHE = {}


def kernel(**inputs) -> np.ndarray:
    value = np.ascontiguousarray(np.asarray(inputs["value"], dtype=np.float32))
    assert value.shape == (T_FULL, N), value.shape

    if "nc" not in _CACHE:
        _CACHE["nc"] = build_bass(T_CORE)
    nc = _CACHE["nc"]

    consts = _host_consts()
    in_maps = [
        {"va": _host_shuffle_in(value[c * T_CORE : (c + 1) * T_CORE], consts)}
        for c in range(N_CORES)
    ]
    try:
        res = run_bass_kernel_spmd(nc, in_maps, list(range(N_CORES)))
        out = np.concatenate(
            [_host_unshuffle_out(r["out"], T_CORE) for r in res.results], axis=0
        )
        BASS_RAN["ok"] = True
        return out
    except Exception as e:
        print(
            f"kernel.py: BASS PATH FAILED ({type(e).__name__}: {e}); "
            "falling back to XLA",
            file=sys.stderr,
        )
        BASS_RAN["ok"] = False
        import jax
        import jax.numpy as jnp

        devs = jax.devices()[:N_CORES]
        scale = np.float32(1.0 / np.sqrt(np.float32(N)))
        w = np.asarray(inputs["weight"], dtype=np.float32)
        outs = []
        for c in range(N_CORES):
            d = devs[c % len(devs)]
            f = jax.jit(lambda a, b: jnp.dot(a, b) * scale, device=d)
            outs.append(f(value[c * T_CORE : (c + 1) * T_CORE], w))
        return np.concatenate([np.asarray(o) for o in outs], axis=0).astype(
            np.float32
        )


# revision 22
# speedup vs baseline: 1.0564x; 1.0564x over previous
"""Trainium2 Bass kernel: 2048-point Hadamard transform.

out = (value @ H2048) * 2^-5.5,  value: (32768, 2048) f32, H2048 = Sylvester
Hadamard (symmetric, +-1) == the `weight` input.

Factorization: H2048 = H128 (x) H16 over n = 16*i + k, m = 16*j + l:
    out[t, 16j+l] = sum_k H16[k,l] * (sum_i V[t,16i+k] * H128[i,j])

Zero on-device transposes:
  * Host pre-shuffles input to va[i, (t,k)] (i on partitions), bf16.
  * Stage A is DATA-stationary: lhsT = va chunk [i, (t8,k)], rhs = H128s
    -> psA[(t8,k), j]: the layout change falls out of lhsT.T @ rhs for free.
  * Stage B is H-stationary with block-diagonal R1 = I8 (x) H16 acting on
    partitions (t8,k) -> psB[(t8,l), (c,j)].
  * Host un-shuffles the [(t8,l), (it,c,j)] output back to [t, m].

Scales: H128s = H128 * 2^-3 (exact bf16), R1 = (I8 (x) H16) * 2^-2.5;
product = 2^-5.5 = 1/sqrt(2048).

walrus in this container rejects any instruction carrying more than ONE
sync wait (one NEURON_ISA_TPB_EVENTS slot per instruction), while Tile
freely emits several (slot-reuse WAR/WAW + cross-engine RAW on one inst,
and the kernel-tail drain waits on every live semaphore). _legalize_waits
post-processes the BIR JSON: every excess wait moves onto an injected
same-engine EventSemaphore placed immediately before the instruction --
the engine stalls for the same conditions in sequence, so the schedule
semantics are unchanged.

Sharding: data-parallel on tokens across 8 cores (4096 tokens each);
bf16 in / bf16 out DMA (32 MB per core), host casts f32<->bf16.
PSUM evacuation is split DVE (stage A) / ACT (stage B).
"""

import json
import sys

import numpy as np
import ml_dtypes

import concourse.bass as bass
import concourse.mybir as mybir
import concourse.tile as tile
from concourse.bass import ts, _add_dep_helper as add_dep
from concourse.bass_utils import run_bass_kernel_spmd

N_CORES = 8
T_FULL = 32768
N = 2048
T_CORE = T_FULL // N_CORES  # 4096
P = 128
N_TILES = T_CORE // P  # 32
# Variable DMA granularity (in 512KB token tiles): small chunks at the
# edges for fast ramp-in/ramp-out, 4MB chunks mid-kernel for efficiency.
LOAD_GRPS = [1, 1, 2, 4, 4, 4, 4, 4, 4, 4]
STORE_GRPS = [2, 4, 4, 4, 4, 4, 4, 4, 1, 1]
CW = 2 * P  # consts width appended to the first load

F32 = mybir.dt.float32
BF16 = mybir.dt.bfloat16
NP_BF16 = ml_dtypes.bfloat16

BASS_RAN = {"ok": False}  # introspection for test.py


def _sylvester(n: int) -> np.ndarray:
    H = np.array([[1.0]], dtype=np.float64)
    while H.shape[0] < n:
        H = np.block([[H, H], [H, -H]])
    return H


def _host_consts() -> np.ndarray:
    H128s = _sylvester(128) * 2.0**-3
    R1 = np.kron(np.eye(8), _sylvester(16)) * 2.0**-2.5
    return np.concatenate([H128s, R1], axis=1).astype(NP_BF16)  # [128, 256]


def _legalize_waits(bir: bytes) -> bytes:
    """Split multi-wait instructions for walrus's 1-wait-per-inst limit.

    Each excess wait moves to an injected same-engine EventSemaphore right
    before the instruction; the engine blocks on the same conditions in
    the same order, so this is semantics-preserving.
    """
    j = json.loads(bir)
    n = 0
    for f in j["functions"]:
        for b in f["blocks"]:
            out = []
            for i in b["instructions"]:
                si = i.get("sync_info")
                if si and si.get("on_wait") and len(si["on_wait"]) > 1:
                    waits = si["on_wait"]
                    for w in waits[:-1]:
                        n += 1
                        out.append(
                            {
                                "debug": i.get("debug", 0),
                                "engine": i["engine"],
                                "ins": [],
                                "outs": [],
                                "name": f"lgw-{n}-{i['name']}",
                                "opcode": "EventSemaphore",
                                "sync_info": {"on_update": [], "on_wait": [w]},
                            }
                        )
                    si["on_wait"] = [waits[-1]]
                out.append(i)
            b["instructions"] = out
    return json.dumps(j).encode()


def build_bass(t_core: int = T_CORE) -> bass.Bass:
    n_tiles = t_core // P
    assert sum(LOAD_GRPS) == n_tiles and sum(STORE_GRPS) == n_tiles
    load_start = {}
    s = 0
    for k, w in enumerate(LOAD_GRPS):
        load_start[s] = (k, w)
        s += w
    store_start = {}
    s = 0
    for k, w in enumerate(STORE_GRPS):
        store_start[s] = (k, w)
        s += w
    max_lw = max(LOAD_GRPS[1:])
    max_sw = max(STORE_GRPS)

    nc = bass.Bass()
    # the first load carries the consts in its last CW columns
    va_p = nc.declare_dram_parameter(
        "va", [P, t_core * 16 + CW], BF16, isOutput=False
    )
    out_p = nc.declare_dram_parameter("out", [P, t_core * 16], BF16, isOutput=True)

    with tile.TileContext(nc) as tc:
        with (
            tc.tile_pool(name="va0", bufs=1) as va0pool,
            tc.tile_pool(name="va", bufs=3) as vapool,
            tc.tile_pool(name="vt2", bufs=4) as vtpool,
            tc.tile_pool(name="og", bufs=3) as ogpool,
            tc.tile_pool(name="pa", bufs=2, space="PSUM") as papool,
            tc.tile_pool(name="pb", bufs=2, space="PSUM") as pbpool,
        ):
            H128s = R1 = None
            VA = OG = None
            va_base = og_base = 0
            vt2_of = {}
            pe_prev = [None]

            def pe_chain(ins):
                # pin PE issue order (software pipeline: stage A of iter i
                # before stage B of iter i-1) without adding semaphores
                if pe_prev[0] is not None:
                    add_dep(ins.ins, pe_prev[0].ins, sync=False)
                pe_prev[0] = ins

            def emit_stage_b(j):
                # stage B for tile j: psB[(t8,l), (c,j)] = R1.T @ vt2
                _, sw_ = store_start[og_base_of[j]]
                ob = (j - og_base_of[j]) * N
                OGj = og_of[j]
                vt2j = vt2_of.pop(j)
                for half in range(2):
                    pb = pbpool.tile([P, 2, 512], F32, tag="pb")  # 2 banks
                    for e in range(2):
                        gq = half * 2 + e
                        mm = nc.tensor.matmul(
                            pb[:, e],
                            R1,
                            vt2j[:, ts(gq, 4), :],
                            start=True,
                            stop=True,
                        )
                        pe_chain(mm)
                    # evacuate + cast (ACT)
                    nc.scalar.copy(
                        out=OGj[:, ob + half * 1024 : ob + (half + 1) * 1024],
                        in_=pb,
                    )
                if j == og_base_of[j] + sw_ - 1:
                    # store on the ACT HWDGE ring; loads use the SP ring
                    nc.scalar.dma_start(
                        out=out_p[:, og_base_of[j] * N : (og_base_of[j] + sw_) * N],
                        in_=OGj[:, 0 : sw_ * N],
                    )

            og_of = {}
            og_base_of = {}
            for it in range(n_tiles):
                if it in load_start:
                    k, w = load_start[it]
                    va_base = it
                    if k == 0:
                        fw0 = w * N
                        VA = va0pool.tile([P, fw0 + CW], BF16, tag="va0")
                        nc.sync.dma_start(out=VA, in_=va_p[:, 0 : fw0 + CW])
                        H128s = VA[:, fw0 : fw0 + P]
                        R1 = VA[:, fw0 + P : fw0 + 2 * P]
                    else:
                        VA = vapool.tile([P, max_lw * N], BF16, tag="va")
                        nc.sync.dma_start(
                            out=VA[:, 0 : w * N],
                            in_=va_p[:, CW + it * N : CW + (it + w) * N],
                        )
                if it in store_start:
                    _, sw = store_start[it]
                    og_base = it
                    OG = ogpool.tile([P, max_sw * N], BF16, tag="og")
                og_of[it] = OG
                og_base_of[it] = og_base
                base = (it - va_base) * N
                vt2 = vtpool.tile([P, 16, P], BF16, tag="vt2")
                vt2_of[it] = vt2
                # stage A: psA[(t8,k), j] = sum_i va[i,(t8,k)] H128s[i,j]
                for h in range(2):
                    pa = papool.tile([P, 8, P], F32, tag="pa")  # 2 banks
                    for cc in range(8):
                        c = h * 8 + cc
                        mm = nc.tensor.matmul(
                            pa[:, cc],
                            VA[:, base + c * P : base + (c + 1) * P],
                            H128s,
                            start=True,
                            stop=True,
                        )
                        pe_chain(mm)
                    # evacuate + cast f32->bf16 (DVE)
                    nc.vector.tensor_copy(
                        out=vt2[:, h * 8 : (h + 1) * 8, :], in_=pa
                    )
                if it > 0:
                    emit_stage_b(it - 1)
            emit_stage_b(n_tiles - 1)

    # walrus 1-wait workaround: serve a legalized BIR to every consumer
    legal = _legalize_waits(mybir.module_to_json_bytes(nc.m))
    nc.to_json_bytes = lambda: legal
    return nc


def scan_multiwait(nc) -> list:
    bad = []
    for b in nc.m.functions[0].blocks:
        for i in b.instructions:
            si = i.sync_info
            if si and si.on_wait and len(si.on_wait) > 1:
                bad.append(
                    (
                        i.name,
                        type(i).__name__,
                        str(i.engine),
                        [(w.ant_name, w.wait_value) for w in si.on_wait],
                    )
                )
    return bad


def _host_shuffle_in(v: np.ndarray, consts: np.ndarray) -> np.ndarray:
    # [t, 16i+k] f32 -> [i, t*16+k] bf16 (i on partitions); consts spliced
    # in after the first load group's columns
    t = v.shape[0]
    fw0 = LOAD_GRPS[0] * N
    vb = v.astype(NP_BF16)
    va = vb.reshape(t, 128, 16).transpose(1, 0, 2).reshape(128, t * 16)
    return np.ascontiguousarray(
        np.concatenate([va[:, :fw0], consts, va[:, fw0:]], axis=1)
    )


def _host_unshuffle_out(o: np.ndarray, t_core: int) -> np.ndarray:
    # [t8*16+l, it*2048 + c*128 + j] -> [it*128 + c*8 + t8, 16j+l] f32
    n_tiles = t_core // 128
    arr = np.asarray(o).astype(np.float32).reshape(8, 16, n_tiles, 16, 128)
    return arr.transpose(2, 3, 0, 4, 1).reshape(t_core, 2048)


_CACHE = {}


def kernel(**inputs) -> np.ndarray:
    value = np.ascontiguousarray(np.asarray(inputs["value"], dtype=np.float32))
    assert value.shape == (T_FULL, N), value.shape

    if "nc" not in _CACHE:
        _CACHE["nc"] = build_bass(T_CORE)
    nc = _CACHE["nc"]

    consts = _host_consts()
    in_maps = [
        {"va": _host_shuffle_in(value[c * T_CORE : (c + 1) * T_CORE], consts)}
        for c in range(N_CORES)
    ]
    try:
        res = run_bass_kernel_spmd(nc, in_maps, list(range(N_CORES)))
        out = np.concatenate(
            [_host_unshuffle_out(r["out"], T_CORE) for r in res.results], axis=0
        )
        BASS_RAN["ok"] = True
        return out
    except Exception as e:
        print(
            f"kernel.py: BASS PATH FAILED ({type(e).__name__}: {e}); "
            "falling back to XLA",
            file=sys.stderr,
        )
        BASS_RAN["ok"] = False
        import jax
        import jax.numpy as jnp

        devs = jax.devices()[:N_CORES]
        scale = np.float32(1.0 / np.sqrt(np.float32(N)))
        w = np.asarray(inputs["weight"], dtype=np.float32)
        outs = []
        for c in range(N_CORES):
            d = devs[c % len(devs)]
            f = jax.jit(lambda a, b: jnp.dot(a, b) * scale, device=d)
            outs.append(f(value[c * T_CORE : (c + 1) * T_CORE], w))
        return np.concatenate([np.asarray(o) for o in outs], axis=0).astype(
            np.float32
        )


# revision 23
# speedup vs baseline: 1.1172x; 1.0576x over previous
"""Trainium2 Bass kernel: 2048-point Hadamard transform.

out = (value @ H2048) * 2^-5.5,  value: (32768, 2048) f32, H2048 = Sylvester
Hadamard (symmetric, +-1) == the `weight` input.

Factorization: H2048 = H128 (x) H16 over n = 16*i + k, m = 16*j + l:
    out[t, 16j+l] = sum_k H16[k,l] * (sum_i V[t,16i+k] * H128[i,j])

Zero on-device transposes:
  * Host pre-shuffles input to va[i, (t,k)] (i on partitions), bf16.
  * Stage A is DATA-stationary: lhsT = va chunk [i, (t8,k)], rhs = H128s
    -> psA[(t8,k), j]: the layout change falls out of lhsT.T @ rhs for free.
  * Stage B is H-stationary with block-diagonal R1 = I8 (x) H16 acting on
    partitions (t8,k) -> psB[(t8,l), (c,j)].
  * Host un-shuffles the [(t8,l), (it,c,j)] output back to [t, m].

Scales: H128s = H128 * 2^-3 (exact bf16), R1 = (I8 (x) H16) * 2^-2.5;
product = 2^-5.5 = 1/sqrt(2048).

walrus in this container rejects any instruction carrying more than ONE
sync wait (one NEURON_ISA_TPB_EVENTS slot per instruction), while Tile
freely emits several (slot-reuse WAR/WAW + cross-engine RAW on one inst,
and the kernel-tail drain waits on every live semaphore). _legalize_waits
post-processes the BIR JSON: every excess wait moves onto an injected
same-engine EventSemaphore placed immediately before the instruction --
the engine stalls for the same conditions in sequence, so the schedule
semantics are unchanged.

Sharding: data-parallel on tokens across 8 cores (4096 tokens each);
bf16 in / bf16 out DMA (32 MB per core), host casts f32<->bf16.
PSUM evacuation is split DVE (stage A) / ACT (stage B).
"""

import json
import sys

import numpy as np
import ml_dtypes

import concourse.bass as bass
import concourse.mybir as mybir
import concourse.tile as tile
from concourse.bass import ts, _add_dep_helper as add_dep
from concourse.bass_utils import run_bass_kernel_spmd

N_CORES = 8
T_FULL = 32768
N = 2048
T_CORE = T_FULL // N_CORES  # 4096
P = 128
N_TILES = T_CORE // P  # 32
# Variable DMA granularity (in 512KB token tiles): small chunks at the
# edges for fast ramp-in/ramp-out, 4MB chunks mid-kernel for efficiency.
LOAD_GRPS = [1, 1, 2, 4, 4, 4, 4, 4, 4, 4]
STORE_GRPS = [2, 4, 4, 4, 4, 4, 4, 4, 1, 1]
CW = 2 * P  # consts width appended to the first load

F32 = mybir.dt.float32
BF16 = mybir.dt.bfloat16
NP_BF16 = ml_dtypes.bfloat16

BASS_RAN = {"ok": False}  # introspection for test.py


def _sylvester(n: int) -> np.ndarray:
    H = np.array([[1.0]], dtype=np.float64)
    while H.shape[0] < n:
        H = np.block([[H, H], [H, -H]])
    return H


def _host_consts() -> np.ndarray:
    H128s = _sylvester(128) * 2.0**-3
    R1 = np.kron(np.eye(8), _sylvester(16)) * 2.0**-2.5
    return np.concatenate([H128s, R1], axis=1).astype(NP_BF16)  # [128, 256]


def _legalize_waits(bir: bytes) -> bytes:
    """Split multi-wait instructions for walrus's 1-wait-per-inst limit.

    Each excess wait moves to an injected same-engine EventSemaphore right
    before the instruction; the engine blocks on the same conditions in
    the same order, so this is semantics-preserving.
    """
    j = json.loads(bir)
    n = 0
    for f in j["functions"]:
        for b in f["blocks"]:
            out = []
            for i in b["instructions"]:
                si = i.get("sync_info")
                if si and si.get("on_wait") and len(si["on_wait"]) > 1:
                    waits = si["on_wait"]
                    for w in waits[:-1]:
                        n += 1
                        out.append(
                            {
                                "debug": i.get("debug", 0),
                                "engine": i["engine"],
                                "ins": [],
                                "outs": [],
                                "name": f"lgw-{n}-{i['name']}",
                                "opcode": "EventSemaphore",
                                "sync_info": {"on_update": [], "on_wait": [w]},
                            }
                        )
                    si["on_wait"] = [waits[-1]]
                out.append(i)
            b["instructions"] = out
    return json.dumps(j).encode()


def build_bass(t_core: int = T_CORE) -> bass.Bass:
    n_tiles = t_core // P
    assert sum(LOAD_GRPS) == n_tiles and sum(STORE_GRPS) == n_tiles
    load_start = {}
    s = 0
    for k, w in enumerate(LOAD_GRPS):
        load_start[s] = (k, w)
        s += w
    store_start = {}
    s = 0
    for k, w in enumerate(STORE_GRPS):
        store_start[s] = (k, w)
        s += w
    max_lw = max(LOAD_GRPS[1:])
    max_sw = max(STORE_GRPS)

    nc = bass.Bass()
    # the first load carries the consts in its last CW columns
    va_p = nc.declare_dram_parameter(
        "va", [P, t_core * 16 + CW], BF16, isOutput=False
    )
    out_p = nc.declare_dram_parameter("out", [P, t_core * 16], BF16, isOutput=True)

    with tile.TileContext(nc) as tc:
        with (
            tc.tile_pool(name="va0", bufs=1) as va0pool,
            tc.tile_pool(name="va", bufs=6) as vapool,
            tc.tile_pool(name="vt2", bufs=6) as vtpool,
            tc.tile_pool(name="og", bufs=4) as ogpool,
            tc.tile_pool(name="pa", bufs=2, space="PSUM") as papool,
            tc.tile_pool(name="pb", bufs=2, space="PSUM") as pbpool,
        ):
            H128s = R1 = None
            VA = OG = None
            va_base = og_base = 0
            vt2_of = {}
            pe_prev = [None]

            def pe_chain(ins):
                # pin PE issue order (software pipeline: stage A of iter i
                # before stage B of iter i-1) without adding semaphores
                if pe_prev[0] is not None:
                    add_dep(ins.ins, pe_prev[0].ins, sync=False)
                pe_prev[0] = ins

            def emit_stage_b(j):
                # stage B for tile j: psB[(t8,l), (c,j)] = R1.T @ vt2
                _, sw_ = store_start[og_base_of[j]]
                ob = (j - og_base_of[j]) * N
                OGj = og_of[j]
                vt2j = vt2_of.pop(j)
                for half in range(2):
                    pb = pbpool.tile([P, 2, 512], F32, tag="pb")  # 2 banks
                    for e in range(2):
                        gq = half * 2 + e
                        mm = nc.tensor.matmul(
                            pb[:, e],
                            R1,
                            vt2j[:, ts(gq, 4), :],
                            start=True,
                            stop=True,
                        )
                        pe_chain(mm)
                    # evacuate + cast (ACT)
                    nc.scalar.copy(
                        out=OGj[:, ob + half * 1024 : ob + (half + 1) * 1024],
                        in_=pb,
                    )
                if j == og_base_of[j] + sw_ - 1:
                    # store on the ACT HWDGE ring; loads use the SP ring
                    nc.scalar.dma_start(
                        out=out_p[:, og_base_of[j] * N : (og_base_of[j] + sw_) * N],
                        in_=OGj[:, 0 : sw_ * N],
                    )

            og_of = {}
            og_base_of = {}
            for it in range(n_tiles):
                if it in load_start:
                    k, w = load_start[it]
                    va_base = it
                    if k == 0:
                        fw0 = w * N
                        VA = va0pool.tile([P, fw0 + CW], BF16, tag="va0")
                        nc.sync.dma_start(out=VA, in_=va_p[:, 0 : fw0 + CW])
                        H128s = VA[:, fw0 : fw0 + P]
                        R1 = VA[:, fw0 + P : fw0 + 2 * P]
                    else:
                        VA = vapool.tile([P, max_lw * N], BF16, tag="va")
                        nc.sync.dma_start(
                            out=VA[:, 0 : w * N],
                            in_=va_p[:, CW + it * N : CW + (it + w) * N],
                        )
                if it in store_start:
                    _, sw = store_start[it]
                    og_base = it
                    OG = ogpool.tile([P, max_sw * N], BF16, tag="og")
                og_of[it] = OG
                og_base_of[it] = og_base
                base = (it - va_base) * N
                vt2 = vtpool.tile([P, 16, P], BF16, tag="vt2")
                vt2_of[it] = vt2
                # stage A: psA[(t8,k), j] = sum_i va[i,(t8,k)] H128s[i,j]
                for h in range(2):
                    pa = papool.tile([P, 8, P], F32, tag="pa")  # 2 banks
                    for cc in range(8):
                        c = h * 8 + cc
                        mm = nc.tensor.matmul(
                            pa[:, cc],
                            VA[:, base + c * P : base + (c + 1) * P],
                            H128s,
                            start=True,
                            stop=True,
                        )
                        pe_chain(mm)
                    # evacuate + cast f32->bf16 (DVE)
                    nc.vector.tensor_copy(
                        out=vt2[:, h * 8 : (h + 1) * 8, :], in_=pa
                    )
                if it > 0:
                    emit_stage_b(it - 1)
            emit_stage_b(n_tiles - 1)

    # walrus 1-wait workaround: serve a legalized BIR to every consumer
    legal = _legalize_waits(mybir.module_to_json_bytes(nc.m))
    nc.to_json_bytes = lambda: legal
    return nc


def scan_multiwait(nc) -> list:
    bad = []
    for b in nc.m.functions[0].blocks:
        for i in b.instructions:
            si = i.sync_info
            if si and si.on_wait and len(si.on_wait) > 1:
                bad.append(
                    (
                        i.name,
                        type(i).__name__,
                        str(i.engine),
                        [(w.ant_name, w.wait_value) for w in si.on_wait],
                    )
                )
    return bad


def _host_shuffle_in(v: np.ndarray, consts: np.ndarray) -> np.ndarray:
    # [t, 16i+k] f32 -> [i, t*16+k] bf16 (i on partitions); consts spliced
    # in after the first load group's columns
    t = v.shape[0]
    fw0 = LOAD_GRPS[0] * N
    vb = v.astype(NP_BF16)
    va = vb.reshape(t, 128, 16).transpose(1, 0, 2).reshape(128, t * 16)
    return np.ascontiguousarray(
        np.concatenate([va[:, :fw0], consts, va[:, fw0:]], axis=1)
    )


def _host_unshuffle_out(o: np.ndarray, t_core: int) -> np.ndarray:
    # [t8*16+l, it*2048 + c*128 + j] -> [it*128 + c*8 + t8, 16j+l] f32
    n_tiles = t_core // 128
    arr = np.asarray(o).astype(np.float32).reshape(8, 16, n_tiles, 16, 128)
    return arr.transpose(2, 3, 0, 4, 1).reshape(t_core, 2048)


_CACHE = {}


def kernel(**inputs) -> np.ndarray:
    value = np.ascontiguousarray(np.asarray(inputs["value"], dtype=np.float32))
    assert value.shape == (T_FULL, N), value.shape

    if "nc" not in _CACHE:
        _CACHE["nc"] = build_bass(T_CORE)
    nc = _CACHE["nc"]

    consts = _host_consts()
    in_maps = [
        {"va": _host_shuffle_in(value[c * T_CORE : (c + 1) * T_CORE], consts)}
        for c in range(N_CORES)
    ]
    try:
        res = run_bass_kernel_spmd(nc, in_maps, list(range(N_CORES)))
        out = np.concatenate(
            [_host_unshuffle_out(r["out"], T_CORE) for r in res.results], axis=0
        )
        BASS_RAN["ok"] = True
        return out
    except Exception as e:
        print(
            f"kernel.py: BASS PATH FAILED ({type(e).__name__}: {e}); "
            "falling back to XLA",
            file=sys.stderr,
        )
        BASS_RAN["ok"] = False
        import jax
        import jax.numpy as jnp

        devs = jax.devices()[:N_CORES]
        scale = np.float32(1.0 / np.sqrt(np.float32(N)))
        w = np.asarray(inputs["weight"], dtype=np.float32)
        outs = []
        for c in range(N_CORES):
            d = devs[c % len(devs)]
            f = jax.jit(lambda a, b: jnp.dot(a, b) * scale, device=d)
            outs.append(f(value[c * T_CORE : (c + 1) * T_CORE], w))
        return np.concatenate([np.asarray(o) for o in outs], axis=0).astype(
            np.float32
        )
